# revision 1
# baseline (speedup 1.0000x reference)
"""Trainium2 Bass kernel for MultiHeadAttention (B=4, S=1024, E=1024, H=16, Dh=64).

Sharding: 8 cores = (batch b in 0..3) x (head-group hg in 0..1, 8 heads each).
The reference reshapes [B,H,S,Dh] -> [B,S,E] WITHOUT transposing heads back, so
head h's attention output occupies output rows t' = h*64 + s//16 — the final
projection is row-parallel across head groups: no cross-core communication.

Per-core pipeline (all matmuls fp32r = full-rate PE, ~1e-4 rel err):
  XT [e,s] (host-transposed) -> Q^T,K^T [d,s], V [t,d] (+ones cols for sums)
  scores^T[t,s] = K @ Q^T (causal blocks skipped), exp(0.5*x) on ACT (no
  max-subtraction: |scores|<~30 for this distribution, exp is fp32-safe),
  causal masking via gpsimd affine_select (fill 0 post-exp),
  z^T+sums = [V|1]^T @ expSt accumulated in PSUM, normalize fused into the
  scrambled-reshape gather (strided DVE mul by 1/sums), out = X2 @ wo.
"""
import numpy as np

B, S, E, H, DH = 4, 1024, 1024, 16, 64
NCORES = 8
HPC = 8          # heads per core
EC = 8           # 128-row chunks of E
TT = 8           # 128-row t-tiles of S
NJ = 2           # 512-col s-blocks

_CACHE = {}


def _build(variant):
    import concourse.bacc as bacc
    import concourse.tile as tile
    import concourse.mybir as mybir

    f32 = mybir.dt.float32
    f32r = mybir.dt.float32r
    Exp = mybir.ActivationFunctionType.Exp
    mult = mybir.AluOpType.mult
    is_ge = mybir.AluOpType.is_ge

    causal = variant == "causal"

    def computed(tt, j):
        # scores^T block (t-tile tt, s-block j) skipped iff fully masked (t > s)
        if not causal:
            return True
        return 128 * tt <= 512 * j + 511

    def partial(tt, j):
        # block intersects the diagonal -> needs the triangular fill
        return causal and 128 * tt - 1 < 512 * j + 511 and not 128 * tt + 127 <= 512 * j

    nc = bacc.Bacc("TRN2")
    xt = nc.dram_tensor("xt", [128, EC, S], f32r, kind="ExternalInput")
    wq = nc.dram_tensor("wq", [128, EC, 512], f32r, kind="ExternalInput")
    wk = nc.dram_tensor("wk", [128, EC, 512], f32r, kind="ExternalInput")
    wv = nc.dram_tensor("wv", [128, EC, 512], f32r, kind="ExternalInput")
    wo = nc.dram_tensor("wo", [128, EC, 1024], f32r, kind="ExternalInput")
    bq = nc.dram_tensor("bq", [128, 4], f32, kind="ExternalInput")
    bk = nc.dram_tensor("bk", [128, 4], f32, kind="ExternalInput")
    if not causal:
        mkt = nc.dram_tensor("mkt", [128, TT, S], f32, kind="ExternalInput")
    out = nc.dram_tensor("out", [4, 128, 1024], f32, kind="ExternalOutput")

    with tile.TileContext(nc) as tc:
        with (
            tc.tile_pool(name="persist", bufs=1) as pp,
            tc.tile_pool(name="mm", bufs=3, space="PSUM") as mm,
            tc.tile_pool(name="ztp", bufs=2, space="PSUM") as ztp,
        ):
            p1 = tc.alloc_tile_pool(name="p1", bufs=1)
            xt_sb = p1.tile([128, EC, S], f32r)
            wq_sb = p1.tile([128, EC, 512], f32r)
            wk_sb = p1.tile([128, EC, 512], f32r)
            wv_sb = p1.tile([128, EC, 512], f32r)
            for k in range(0, EC, 2):
                nc.sync.dma_start(out=xt_sb[:, k:k + 2, :], in_=xt[:, k:k + 2, :])
                nc.sync.dma_start(out=wq_sb[:, k:k + 2, :], in_=wq[:, k:k + 2, :])
                nc.sync.dma_start(out=wk_sb[:, k:k + 2, :], in_=wk[:, k:k + 2, :])
                nc.sync.dma_start(out=wv_sb[:, k:k + 2, :], in_=wv[:, k:k + 2, :])
            qt_sb = pp.tile([128, 4, S], f32r)
            kt_sb = pp.tile([128, 4, S], f32r)
            vp_sb = pp.tile([128, TT, 1024], f32r)
            x2t_sb = pp.tile([128, EC, 512], f32r)
            bq_sb = pp.tile([128, 4], f32)
            bk_sb = pp.tile([128, 4], f32)
            if not causal:
                mkt_sb = pp.tile([128, TT, S], f32)
                nc.sync.dma_start(out=mkt_sb, in_=mkt.ap())
            nc.sync.dma_start(out=bq_sb, in_=bq.ap())
            nc.sync.dma_start(out=bk_sb, in_=bk.ap())

            # ones columns of V' (cols 64:128 of each head block)
            vview = vp_sb.rearrange("p t (h two d) -> p t h two d", two=2, d=DH)
            ones_sb = pp.tile([128, 512], f32)
            nc.vector.memset(ones_sb, 1.0)
            ones_v = ones_sb.rearrange("p (h d) -> p h d", d=DH)
            for tt in range(TT):
                nc.vector.tensor_copy(vview[:, tt, :, 1, :], ones_v)

            # ---- Q^T / K^T projections: out [d-tile, s] = w.T @ X^T ----
            for wsb, dest, bias in ((wq_sb, qt_sb, bq_sb), (wk_sb, kt_sb, bk_sb)):
                for dt_ in range(4):
                    for sh in range(2):
                        ps = mm.tile([128, 512], f32, tag="mm")
                        for ec in range(EC):
                            nc.tensor.matmul(
                                ps, wsb[:, ec, 128 * dt_:128 * dt_ + 128],
                                xt_sb[:, ec, 512 * sh:512 * sh + 512],
                                start=(ec == 0), stop=(ec == EC - 1),
                            )
                        nc.vector.tensor_scalar_add(
                            out=dest[:, dt_, 512 * sh:512 * sh + 512],
                            in0=ps, scalar1=bias[:, dt_:dt_ + 1],
                        )
            # ---- V projection: out [t-tile, d] = X^T.T @ wv ----
            for tt in range(TT):
                ps = mm.tile([128, 512], f32, tag="mm")
                for ec in range(EC):
                    nc.tensor.matmul(
                        ps, xt_sb[:, ec, 128 * tt:128 * tt + 128],
                        wv_sb[:, ec, :],
                        start=(ec == 0), stop=(ec == EC - 1),
                    )
                nc.vector.tensor_copy(
                    vview[:, tt, :, 0, :], ps.rearrange("p (h d) -> p h d", d=DH)
                )
            p1.release()
            # late pools reuse p1's freed space
            late = tc.alloc_tile_pool(name="late", bufs=1)
            expa = tc.alloc_tile_pool(name="expa", bufs=8)
            expb = tc.alloc_tile_pool(name="expb", bufs=8)
            small = tc.alloc_tile_pool(name="small", bufs=2)
            outp = tc.alloc_tile_pool(name="outp", bufs=2)
            wo_sb = late.tile([128, EC, 1024], f32r)
            nc.sync.dma_start(out=wo_sb, in_=wo.ap())

            # ---- attention, head pairs interleaved ----
            # even head h0 lives on partitions 0-63, odd h1 on 64-127; adjacent
            # scores matmuls auto-derive PE tile_position row-groups and run
            # concurrently in the array (K=64 each, disjoint rows).
            for hp in range(HPC // 2):
                pair = (2 * hp, 2 * hp + 1)
                et = {}
                for tt in range(TT):
                    js = [j for j in range(NJ) if computed(tt, j)]
                    s0 = 512 * js[0]
                    c0 = 128 * tt
                    pss = {}
                    for h in pair:
                        dt_ = h // 2
                        pb = 64 * (h % 2)
                        ps = mm.tile([128, 1024], f32, tag="mm", name=f"ps_{h}_{tt}")
                        pss[h] = ps
                        for j in js:
                            lo = max(512 * j, c0) if causal else 512 * j
                            nc.tensor.matmul(
                                ps[:, lo - s0:512 * j + 512 - s0],
                                kt_sb[pb:pb + 64, dt_, c0:c0 + 128],
                                qt_sb[pb:pb + 64, dt_, lo:512 * j + 512],
                                start=True, stop=True,
                            )
                            if not causal:
                                o = 512 * j - s0
                                nc.vector.tensor_add(
                                    ps[:, o:o + 512],
                                    ps[:, o:o + 512],
                                    mkt_sb[:, tt, 512 * j:512 * j + 512],
                                )
                    for h in pair:
                        ps = pss[h]
                        if tt < 4 or not causal:
                            e = expa.tile([128, 1024], f32r, tag="expa",
                                          name=f"e_{h}_{tt}")
                        else:
                            e = expb.tile([128, 512], f32r, tag="expb",
                                          name=f"e_{h}_{tt}")
                        if causal:
                            # e columns are true s minus s0
                            nc.scalar.activation(
                                e[:, c0 - s0:], ps[:, c0 - s0:1024 - s0],
                                Exp, scale=0.5,
                            )
                            nc.gpsimd.affine_select(
                                out=e[:, 0:c0 + 128 - s0], in_=e[:, 0:c0 + 128 - s0],
                                pattern=[[1, c0 + 128 - s0]], compare_op=is_ge,
                                fill=0.0, base=s0 - c0, channel_multiplier=-1,
                            )
                        else:
                            nc.scalar.activation(
                                e[:, :], ps[:, :1024 - s0], Exp, scale=0.5
                            )
                        for j in js:
                            et[(h, tt, j)] = e[:, 512 * j - s0:512 * j - s0 + 512]
                for h in pair:
                    zt_f = small.tile([64, S], f32, tag="ztf", name=f"ztf_{h}")
                    rec = small.tile([64, S], f32, tag="rec", name=f"rec_{h}")
                    for j in range(NJ):
                        zt = ztp.tile([128, 512], f32, tag="zt", name=f"zt_{h}_{j}")
                        ks = [tt for tt in range(TT) if (h, tt, j) in et]
                        for i, tt in enumerate(ks):
                            lo = max(0, 128 * tt - 512 * j) if causal else 0
                            nc.tensor.matmul(
                                zt[:, lo:], vp_sb[:, tt, 128 * h:128 * h + 128],
                                et[(h, tt, j)][:, lo:],
                                start=(i == 0), stop=(i == len(ks) - 1),
                            )
                        nc.vector.reciprocal(rec[:, 512 * j:512 * j + 512],
                                             zt[64:128, :])
                        nc.vector.tensor_copy(zt_f[:, 512 * j:512 * j + 512],
                                              zt[0:64, :])
                    zv = zt_f.rearrange("p (m c par) -> par p c m", m=64, c=8, par=2)
                    rv = rec.rearrange("p (m c par) -> par p c m", m=64, c=8, par=2)
                    for P in range(2):
                        nc.vector.tensor_tensor(
                            x2t_sb[64 * P:64 * P + 64, :, 64 * h:64 * h + 64],
                            zv[P], rv[P], op=mult,
                        )

            # ---- output projection: out [t'-tile, e'] = X2T.T @ wo ----
            for tp in range(4):
                osb = outp.tile([128, 1024], f32, tag="osb")
                for eh in range(2):
                    ps = mm.tile([128, 512], f32, tag="mm")
                    for c in range(EC):
                        nc.tensor.matmul(
                            ps, x2t_sb[:, c, 128 * tp:128 * tp + 128],
                            wo_sb[:, c, 512 * eh:512 * eh + 512],
                            start=(c == 0), stop=(c == EC - 1),
                        )
                    nc.vector.tensor_copy(osb[:, 512 * eh:512 * eh + 512], ps)
                nc.sync.dma_start(out=out[tp], in_=osb)
            for p in (outp, small, expb, expa, late):
                p.release()
    nc.compile()
    return nc


def kernel(inputs, mask, wq, bq, wk, bk, wv, bv, wo, bo):
    from concourse.bass_utils import run_bass_kernel_spmd

    x = np.asarray(inputs, dtype=np.float32)
    wq = np.asarray(wq, dtype=np.float32)
    wk = np.asarray(wk, dtype=np.float32)
    wv = np.asarray(wv, dtype=np.float32)
    wo = np.asarray(wo, dtype=np.float32)
    bq = np.asarray(bq, dtype=np.float32)
    bk = np.asarray(bk, dtype=np.float32)
    mask2d = np.asarray(mask, dtype=np.float32).reshape(S, S)
    causal_ref = 1.0 - np.tril(np.ones((S, S), dtype=np.float32))
    causal = bool(np.array_equal(mask2d, causal_ref))
    variant = "causal" if causal else "generic"
    if variant not in _CACHE:
        _CACHE[variant] = _build(variant)
    nc = _CACHE[variant]

    in_maps = []
    for c in range(NCORES):
        b, hg = c // 2, c % 2
        sl = slice(512 * hg, 512 * hg + 512)
        m = {
            "xt": np.ascontiguousarray(x[b].T.reshape(EC, 128, S).transpose(1, 0, 2)),
            "wq": np.ascontiguousarray(wq[:, sl].reshape(EC, 128, 512).transpose(1, 0, 2)),
            "wk": np.ascontiguousarray(wk[:, sl].reshape(EC, 128, 512).transpose(1, 0, 2)),
            "wv": np.ascontiguousarray(wv[:, sl].reshape(EC, 128, 512).transpose(1, 0, 2)),
            "wo": np.ascontiguousarray(wo.reshape(EC, 128, 1024).transpose(1, 0, 2)),
            "bq": np.ascontiguousarray(bq[sl].reshape(4, 128).T),
            "bk": np.ascontiguousarray(bk[sl].reshape(4, 128).T),
        }
        if not causal:
            m["mkt"] = np.ascontiguousarray(
                (mask2d.T * np.float32(-2e9)).reshape(TT, 128, S).transpose(1, 0, 2))
        in_maps.append(m)

    global _last_in_maps
    _last_in_maps = in_maps
    res = run_bass_kernel_spmd(nc, in_maps, core_ids=list(range(NCORES)))
    full = np.empty((B, S, E), dtype=np.float32)
    for c in range(NCORES):
        b, hg = c // 2, c % 2
        full[b, 512 * hg:512 * hg + 512, :] = res.results[c]["out"].reshape(512, 1024)

    # biases bv/bo are zero in this problem; fold in exactly if ever nonzero.
    bv = np.asarray(bv, dtype=np.float32)
    bo = np.asarray(bo, dtype=np.float32)
    if np.any(bv != 0):
        # z_norm[b,h,s,d] += bv[64h+d]  =>  X2 += Bmat  =>  out += Bmat @ wo
        bmat = np.zeros((S, E), dtype=np.float64)
        tpr = np.arange(S)
        e = np.arange(E)
        bmat[:, :] = bv[(64 * (tpr[:, None] // 64) + e[None, :] % 64)]
        full += (bmat @ np.asarray(wo, dtype=np.float64)).astype(np.float32)[None]
    if np.any(bo != 0):
        full += bo[None, None, :]
    return full



# revision 13
# speedup vs baseline: 1.2601x; 1.2601x over previous
"""Trainium2 Bass kernel for MultiHeadAttention (B=4, S=1024, E=1024, H=16, Dh=64).

Sharding: 8 cores = (batch b in 0..3) x (head-group hg in 0..1, 8 heads each).
The reference reshapes [B,H,S,Dh] -> [B,S,E] WITHOUT transposing heads back, so
head h's attention output occupies output rows t' = h*64 + s//16 — the final
projection is row-parallel across head groups: no cross-core communication.

v2 (causal path), tuned against the TimelineSim cost model:
  - bf16 matmul operands everywhere (PSUM accumulates f32): 1 cycle/row at any
    free size, halves DMA bytes and SBUF footprint. f32 path kept for the
    generic-mask fallback.
  - DMA streamed per-128-row contraction chunk (xt[ec]+wq[ec] interleaved
    first) so PE starts ~2.5us in and never waits on weights again.
  - Q/K/V projections ec-outer into 8 concurrent PSUM accumulators (exactly
    the 8 banks), so matmuls chase the DMA stream chunk by chunk.
  - attention runs one head-pair behind scores emission (scores p -> z p-1),
    giving the exp/mask chain a full scores-block of slack; PE never stalls
    on Activation/Pool.
  - softmax division fused: z and the ones-row sums are read straight out of
    PSUM with a strided AP and divided into the scrambled-reshape x2t layout
    (no SBUF staging copies, no reciprocal pass).
  - output projection DMAs straight from PSUM to DRAM (no SBUF bounce).
"""
import numpy as np

B, S, E, H, DH = 4, 1024, 1024, 16, 64
NCORES = 8
HPC = 8          # heads per core
EC = 8           # 128-row chunks of E
TT = 8           # 128-row t-tiles of S
NJ = 2           # 512-col s-blocks

_CACHE = {}


def _build_causal():
    import concourse.bacc as bacc
    import concourse.tile as tile
    import concourse.mybir as mybir

    f32 = mybir.dt.float32
    bf16 = mybir.dt.bfloat16
    Exp = mybir.ActivationFunctionType.Exp
    Copy = mybir.ActivationFunctionType.Copy
    mult = mybir.AluOpType.mult
    is_ge = mybir.AluOpType.is_ge

    nc = bacc.Bacc("TRN2")
    xt = nc.dram_tensor("xt", [128, EC, S], bf16, kind="ExternalInput")
    wq = nc.dram_tensor("wq", [128, EC, 512], bf16, kind="ExternalInput")
    wk = nc.dram_tensor("wk", [128, EC, 512], bf16, kind="ExternalInput")
    wv = nc.dram_tensor("wv", [128, EC, 512], bf16, kind="ExternalInput")
    wo = nc.dram_tensor("wo", [128, EC, 1024], bf16, kind="ExternalInput")
    bq = nc.dram_tensor("bq", [128, 4], f32, kind="ExternalInput")
    bk = nc.dram_tensor("bk", [128, 4], f32, kind="ExternalInput")
    out = nc.dram_tensor("out", [4, 128, 1024], f32, kind="ExternalOutput")

    with tile.TileContext(nc) as tc:
        with (
            tc.tile_pool(name="persist", bufs=1) as pp,
        ):
            p1 = tc.alloc_tile_pool(name="p1", bufs=1)
            xt_sb = p1.tile([128, EC, S], bf16)
            wq_sb = p1.tile([128, EC, 512], bf16)
            wk_sb = p1.tile([128, EC, 512], bf16)
            wv_sb = p1.tile([128, EC, 512], bf16)

            qt_sb = pp.tile([128, 4, S], bf16)
            kt_sb = pp.tile([128, 4, S], bf16)
            vp_sb = pp.tile([128, TT, 1024], bf16)
            x2t_sb = pp.tile([128, EC, 512], bf16)
            wo_sb = pp.tile([128, EC, 1024], bf16)
            bq_sb = pp.tile([128, 4], f32)
            bk_sb = pp.tile([128, 4], f32)

            # DMA program order == SP issue order: xt/wq chunk-interleaved so
            # the first Q matmul fires ~4us in, then wk/wv/biases/wo.
            nc.sync.dma_start(out=xt_sb[:, 0, 0:512], in_=xt[:, 0, 0:512])
            nc.sync.dma_start(out=wq_sb[:, 0:1, :], in_=wq[:, 0:1, :])
            nc.sync.dma_start(out=xt_sb[:, 0, 512:1024], in_=xt[:, 0, 512:1024])
            for ec in range(1, EC):
                nc.sync.dma_start(out=xt_sb[:, ec:ec + 1, :], in_=xt[:, ec:ec + 1, :])
                nc.sync.dma_start(out=wq_sb[:, ec:ec + 1, :], in_=wq[:, ec:ec + 1, :])
            for k in range(0, EC, 2):
                nc.sync.dma_start(out=wk_sb[:, k:k + 2, :], in_=wk[:, k:k + 2, :])
            for k in range(0, EC, 2):
                nc.sync.dma_start(out=wv_sb[:, k:k + 2, :], in_=wv[:, k:k + 2, :])
            nc.sync.dma_start(out=bq_sb, in_=bq.ap())
            nc.sync.dma_start(out=bk_sb, in_=bk.ap())
            for k in range(0, EC, 4):
                nc.sync.dma_start(out=wo_sb[:, k:k + 4, :], in_=wo[:, k:k + 4, :])

            # ones columns of V' (cols 64:128 of each head block)
            vview = vp_sb.rearrange("p t (h two d) -> p t h two d", two=2, d=DH)
            ones_sb = pp.tile([128, 512], bf16)
            nc.vector.memset(ones_sb, 1.0)
            ones_v = ones_sb.rearrange("p (h d) -> p h d", d=DH)
            for tt in range(TT):
                nc.vector.tensor_copy(vview[:, tt, :, 1, :], ones_v)

            # ---- Q projection: dt0-2 ec-outer with 6 live accumulators so
            # matmuls chase the xt/wq DMA stream chunk by chunk; dt3 and K go
            # through the separate mm pool (pre-allocated: its banks never
            # alias the Q accumulators, so K starts without waiting on the
            # Q drains) ----
            mm = tc.alloc_tile_pool(name="mm", bufs=2, space="PSUM")
            psQ = tc.alloc_tile_pool(name="psQ", bufs=1, space="PSUM")
            qaccs = [psQ.tile([128, 512], f32, tag=f"qacc{i}", name=f"qacc{i}")
                     for i in range(6)]
            for ec in range(EC):
                for dt_ in range(3):
                    for sh in range(2):
                        nc.tensor.matmul(
                            qaccs[2 * dt_ + sh],
                            wq_sb[:, ec, 128 * dt_:128 * dt_ + 128],
                            xt_sb[:, ec, 512 * sh:512 * sh + 512],
                            start=(ec == 0), stop=(ec == EC - 1),
                        )
            for sh in range(2):
                ps = mm.tile([128, 512], f32, tag="mm", name=f"q3ps_{sh}")
                for ec in range(EC):
                    nc.tensor.matmul(
                        ps, wq_sb[:, ec, 384:512],
                        xt_sb[:, ec, 512 * sh:512 * sh + 512],
                        start=(ec == 0), stop=(ec == EC - 1),
                    )
                nc.vector.tensor_scalar_add(
                    out=qt_sb[:, 3, 512 * sh:512 * sh + 512],
                    in0=ps, scalar1=bq_sb[:, 3:4],
                )
            # ---- K projection: ec-inner (wk fully resident by now).
            # Q-accumulator drains are emitted AFTER each K group's drain so
            # on the in-order DVE queue K's psum rotation is never stuck
            # behind a backlog of Q drains. ----
            qdrains = [
                (lambda d=dt_, s=sh: nc.vector.tensor_scalar_add(
                    out=qt_sb[:, d, 512 * s:512 * s + 512],
                    in0=qaccs[2 * d + s], scalar1=bq_sb[:, d:d + 1]))
                for dt_ in range(3) for sh in range(2)
            ]
            for g, (dt_, sh) in enumerate([(d, s) for d in range(4) for s in range(2)]):
                ps = mm.tile([128, 512], f32, tag="mm", name=f"kps_{dt_}_{sh}")
                for ec in range(EC):
                    nc.tensor.matmul(
                        ps, wk_sb[:, ec, 128 * dt_:128 * dt_ + 128],
                        xt_sb[:, ec, 512 * sh:512 * sh + 512],
                        start=(ec == 0), stop=(ec == EC - 1),
                    )
                nc.vector.tensor_scalar_add(
                    out=kt_sb[:, dt_, 512 * sh:512 * sh + 512],
                    in0=ps, scalar1=bk_sb[:, dt_:dt_ + 1],
                )
                if g < len(qdrains):
                    qdrains[g]()
            psQ.release()

            eta = tc.alloc_tile_pool(name="eta", bufs=32)
            outp = tc.alloc_tile_pool(name="outp", bufs=3)
            psS = tc.alloc_tile_pool(name="psS", bufs=3, space="PSUM")

            et = {}

            def emit_s_tile(hp, tt, h):
                # one scores tile [t-tile, s>=c0] -> exp -> causal-mask fill
                c0 = 128 * tt
                pb = 64 * (h % 2)
                ps = psS.tile([128, 1024], f32, tag="s", name=f"ps_{h}_{tt}")
                for j in range(NJ):
                    lo = max(512 * j, c0)
                    if lo >= 512 * j + 512:
                        continue
                    nc.tensor.matmul(
                        ps[:, lo:512 * j + 512],
                        kt_sb[pb:pb + 64, hp, c0:c0 + 128],
                        qt_sb[pb:pb + 64, hp, lo:512 * j + 512],
                        start=True, stop=True,
                    )
                e = eta.tile([128, 1024], bf16, tag="e", name=f"e_{h}_{tt}")
                nc.scalar.activation(e[:, c0:], ps[:, c0:], Exp, scale=0.5)
                nc.gpsimd.affine_select(
                    out=e[:, c0:c0 + 128], in_=e[:, c0:c0 + 128],
                    pattern=[[1, 128]], compare_op=is_ge,
                    fill=0.0, base=0, channel_multiplier=-1,
                )
                et[(h, tt)] = e

            def emit_v_unit(tt):
                ps = mm.tile([128, 512], f32, tag="mm", name=f"vps_{tt}")
                for ec in range(EC):
                    nc.tensor.matmul(
                        ps, xt_sb[:, ec, 128 * tt:128 * tt + 128],
                        wv_sb[:, ec, :],
                        start=(ec == 0), stop=(ec == EC - 1),
                    )
                nc.vector.tensor_copy(
                    vview[:, tt, :, 0, :], ps.rearrange("p (h d) -> p h d", d=DH))

            def emit_z_unit(h, j):
                zt = mm.tile([128, 512], f32, tag="mm", name=f"zt_{h}_{j}")
                ks = [tt for tt in range(TT) if 128 * tt < 512 * j + 512]
                for i, tt in enumerate(ks):
                    lo = max(0, 128 * tt - 512 * j)
                    nc.tensor.matmul(
                        zt[:, lo:], vp_sb[:, tt, 128 * h:128 * h + 128],
                        et[(h, tt)][:, 512 * j + lo:512 * j + 512],
                        start=(i == 0), stop=(i == len(ks) - 1),
                    )
                # x2t[(s%2)*64+d, (s//2)%8, 64h + s//16] = z * (1/sums).
                # DVE may read only one PSUM operand per op: reciprocal the
                # ones-matmul sums (psum rows 64:128) into SBUF, then multiply
                # the z rows straight out of PSUM by it.
                rec = outp.tile([64, 512], f32, tag="rec", name=f"rec_{h}_{j}")
                nc.vector.reciprocal(rec, zt[64:128, :])
                zv = zt.rearrange("p (m c par) -> par p c m", m=32, c=8, par=2)
                rv = rec.rearrange("p (m c par) -> par p c m", m=32, c=8, par=2)
                for P in range(2):
                    nc.vector.tensor_tensor(
                        out=x2t_sb[64 * P:64 * P + 64, :,
                                   64 * h + 32 * j:64 * h + 32 * j + 32],
                        in0=zv[P, 0:64], in1=rv[P], op=mult,
                    )

            def emit_out_unit(tp, eh):
                po = mm.tile([128, 512], f32, tag="mm", name=f"po_{tp}_{eh}")
                for c in range(EC):
                    nc.tensor.matmul(
                        po, x2t_sb[:, c, 128 * tp:128 * tp + 128],
                        wo_sb[:, c, 512 * eh:512 * eh + 512],
                        start=(c == 0), stop=(c == EC - 1),
                    )
                osb = outp.tile([128, 512], f32, tag="osb", name=f"osb_{tp}_{eh}")
                nc.vector.tensor_copy(osb, po)
                nc.sync.dma_start(
                    out=out[tp][:, 512 * eh:512 * eh + 512], in_=osb)

            # ---- pair 0 scores interleaved with V units: keeps PE busy while
            # the (slower) Activation engine chews through the exps ----
            for tt in range(TT):
                emit_s_tile(0, tt, 0)
                emit_s_tile(0, tt, 1)
                emit_v_unit(tt)

            # ---- pairs 1..3: scores(hp) interleaved with z(hp-1)/out(hp-1) ----
            for hp in range(1, HPC // 2):
                ph, pt = 2 * (hp - 1), 2 * (hp - 1) + 1
                fillers = {
                    0: lambda h=ph: emit_z_unit(h, 0),
                    1: lambda h=pt: emit_z_unit(h, 0),
                    2: lambda h=ph: emit_z_unit(h, 1),
                    3: lambda h=pt: emit_z_unit(h, 1),
                    5: lambda t=hp - 1: emit_out_unit(t, 0),
                    7: lambda t=hp - 1: emit_out_unit(t, 1),
                }
                for tt in range(TT):
                    emit_s_tile(hp, tt, 2 * hp)
                    emit_s_tile(hp, tt, 2 * hp + 1)
                    if tt in fillers:
                        fillers[tt]()
            # ---- last pair's z + out, then out(3) ----
            for h in (6, 7):
                for j in range(NJ):
                    emit_z_unit(h, j)
            emit_out_unit(3, 0)
            # final half in two pipelined column strips to shorten the tail
            for q in range(2):
                po = mm.tile([128, 256], f32, tag="mm", name=f"po_3_1_{q}")
                for c in range(EC):
                    nc.tensor.matmul(
                        po, x2t_sb[:, c, 384:512],
                        wo_sb[:, c, 512 + 256 * q:768 + 256 * q],
                        start=(c == 0), stop=(c == EC - 1),
                    )
                osb = outp.tile([128, 256], f32, tag="osb", name=f"osb_3_1_{q}")
                nc.vector.tensor_copy(osb, po)
                nc.sync.dma_start(
                    out=out[3][:, 512 + 256 * q:768 + 256 * q], in_=osb)
            for p in (psS, outp, eta, mm, p1):
                p.release()
    nc.compile()
    return nc


def _build_generic():
    """f32r fallback for a non-causal mask (mask values streamed in)."""
    import concourse.bacc as bacc
    import concourse.tile as tile
    import concourse.mybir as mybir

    f32 = mybir.dt.float32
    f32r = mybir.dt.float32r
    Exp = mybir.ActivationFunctionType.Exp
    mult = mybir.AluOpType.mult

    nc = bacc.Bacc("TRN2")
    xt = nc.dram_tensor("xt", [128, EC, S], f32r, kind="ExternalInput")
    wq = nc.dram_tensor("wq", [128, EC, 512], f32r, kind="ExternalInput")
    wk = nc.dram_tensor("wk", [128, EC, 512], f32r, kind="ExternalInput")
    wv = nc.dram_tensor("wv", [128, EC, 512], f32r, kind="ExternalInput")
    wo = nc.dram_tensor("wo", [128, EC, 1024], f32r, kind="ExternalInput")
    bq = nc.dram_tensor("bq", [128, 4], f32, kind="ExternalInput")
    bk = nc.dram_tensor("bk", [128, 4], f32, kind="ExternalInput")
    mkt = nc.dram_tensor("mkt", [128, TT, S], f32, kind="ExternalInput")
    out = nc.dram_tensor("out", [4, 128, 1024], f32, kind="ExternalOutput")

    with tile.TileContext(nc) as tc:
        with (
            tc.tile_pool(name="persist", bufs=1) as pp,
            tc.tile_pool(name="mm", bufs=3, space="PSUM") as mm,
            tc.tile_pool(name="ztp", bufs=2, space="PSUM") as ztp,
        ):
            p1 = tc.alloc_tile_pool(name="p1", bufs=1)
            xt_sb = p1.tile([128, EC, S], f32r)
            wq_sb = p1.tile([128, EC, 512], f32r)
            wk_sb = p1.tile([128, EC, 512], f32r)
            wv_sb = p1.tile([128, EC, 512], f32r)
            for k in range(0, EC, 2):
                nc.sync.dma_start(out=xt_sb[:, k:k + 2, :], in_=xt[:, k:k + 2, :])
                nc.sync.dma_start(out=wq_sb[:, k:k + 2, :], in_=wq[:, k:k + 2, :])
                nc.sync.dma_start(out=wk_sb[:, k:k + 2, :], in_=wk[:, k:k + 2, :])
                nc.sync.dma_start(out=wv_sb[:, k:k + 2, :], in_=wv[:, k:k + 2, :])
            qt_sb = pp.tile([128, 4, S], f32r)
            kt_sb = pp.tile([128, 4, S], f32r)
            vp_sb = pp.tile([128, TT, 1024], f32r)
            x2t_sb = pp.tile([128, EC, 512], f32r)
            bq_sb = pp.tile([128, 4], f32)
            bk_sb = pp.tile([128, 4], f32)
            mkt_sb = pp.tile([128, TT, S], f32)
            nc.sync.dma_start(out=mkt_sb, in_=mkt.ap())
            nc.sync.dma_start(out=bq_sb, in_=bq.ap())
            nc.sync.dma_start(out=bk_sb, in_=bk.ap())

            vview = vp_sb.rearrange("p t (h two d) -> p t h two d", two=2, d=DH)
            ones_sb = pp.tile([128, 512], f32)
            nc.vector.memset(ones_sb, 1.0)
            ones_v = ones_sb.rearrange("p (h d) -> p h d", d=DH)
            for tt in range(TT):
                nc.vector.tensor_copy(vview[:, tt, :, 1, :], ones_v)

            for wsb, dest, bias in ((wq_sb, qt_sb, bq_sb), (wk_sb, kt_sb, bk_sb)):
                for dt_ in range(4):
                    for sh in range(2):
                        ps = mm.tile([128, 512], f32, tag="mm")
                        for ec in range(EC):
                            nc.tensor.matmul(
                                ps, wsb[:, ec, 128 * dt_:128 * dt_ + 128],
                                xt_sb[:, ec, 512 * sh:512 * sh + 512],
                                start=(ec == 0), stop=(ec == EC - 1),
                            )
                        nc.vector.tensor_scalar_add(
                            out=dest[:, dt_, 512 * sh:512 * sh + 512],
                            in0=ps, scalar1=bias[:, dt_:dt_ + 1],
                        )
            for tt in range(TT):
                ps = mm.tile([128, 512], f32, tag="mm")
                for ec in range(EC):
                    nc.tensor.matmul(
                        ps, xt_sb[:, ec, 128 * tt:128 * tt + 128],
                        wv_sb[:, ec, :],
                        start=(ec == 0), stop=(ec == EC - 1),
                    )
                nc.vector.tensor_copy(
                    vview[:, tt, :, 0, :], ps.rearrange("p (h d) -> p h d", d=DH)
                )
            p1.release()
            late = tc.alloc_tile_pool(name="late", bufs=1)
            expa = tc.alloc_tile_pool(name="expa", bufs=8)
            small = tc.alloc_tile_pool(name="small", bufs=2)
            outp = tc.alloc_tile_pool(name="outp", bufs=2)
            wo_sb = late.tile([128, EC, 1024], f32r)
            nc.sync.dma_start(out=wo_sb, in_=wo.ap())

            for hp in range(HPC // 2):
                pair = (2 * hp, 2 * hp + 1)
                et = {}
                for tt in range(TT):
                    pss = {}
                    for h in pair:
                        dt_ = h // 2
                        pb = 64 * (h % 2)
                        ps = mm.tile([128, 1024], f32, tag="mm", name=f"ps_{h}_{tt}")
                        pss[h] = ps
                        for j in range(NJ):
                            nc.tensor.matmul(
                                ps[:, 512 * j:512 * j + 512],
                                kt_sb[pb:pb + 64, dt_, 128 * tt:128 * tt + 128],
                                qt_sb[pb:pb + 64, dt_, 512 * j:512 * j + 512],
                                start=True, stop=True,
                            )
                            nc.vector.tensor_add(
                                ps[:, 512 * j:512 * j + 512],
                                ps[:, 512 * j:512 * j + 512],
                                mkt_sb[:, tt, 512 * j:512 * j + 512],
                            )
                    for h in pair:
                        e = expa.tile([128, 1024], f32r, tag="expa",
                                      name=f"e_{h}_{tt}")
                        nc.scalar.activation(e[:, :], pss[h][:, :], Exp, scale=0.5)
                        for j in range(NJ):
                            et[(h, tt, j)] = e[:, 512 * j:512 * j + 512]
                for h in pair:
                    zt_f = small.tile([64, S], f32, tag="ztf", name=f"ztf_{h}")
                    rec = small.tile([64, S], f32, tag="rec", name=f"rec_{h}")
                    for j in range(NJ):
                        zt = ztp.tile([128, 512], f32, tag="zt", name=f"zt_{h}_{j}")
                        for i, tt in enumerate(range(TT)):
                            nc.tensor.matmul(
                                zt, vp_sb[:, tt, 128 * h:128 * h + 128],
                                et[(h, tt, j)],
                                start=(i == 0), stop=(i == TT - 1),
                            )
                        nc.vector.reciprocal(rec[:, 512 * j:512 * j + 512],
                                             zt[64:128, :])
                        nc.vector.tensor_copy(zt_f[:, 512 * j:512 * j + 512],
                                              zt[0:64, :])
                    zv = zt_f.rearrange("p (m c par) -> par p c m", m=64, c=8, par=2)
                    rv = rec.rearrange("p (m c par) -> par p c m", m=64, c=8, par=2)
                    for P in range(2):
                        nc.vector.tensor_tensor(
                            x2t_sb[64 * P:64 * P + 64, :, 64 * h:64 * h + 64],
                            zv[P], rv[P], op=mult,
                        )

            for tp in range(4):
                osb = outp.tile([128, 1024], f32, tag="osb")
                for eh in range(2):
                    ps = mm.tile([128, 512], f32, tag="mm")
                    for c in range(EC):
                        nc.tensor.matmul(
                            ps, x2t_sb[:, c, 128 * tp:128 * tp + 128],
                            wo_sb[:, c, 512 * eh:512 * eh + 512],
                            start=(c == 0), stop=(c == EC - 1),
                        )
                    nc.vector.tensor_copy(osb[:, 512 * eh:512 * eh + 512], ps)
                nc.sync.dma_start(out=out[tp], in_=osb)
            for p in (outp, small, expa, late):
                p.release()
    nc.compile()
    return nc


def kernel(inputs, mask, wq, bq, wk, bk, wv, bv, wo, bo):
    import ml_dtypes
    from concourse.bass_utils import run_bass_kernel_spmd

    bf16 = ml_dtypes.bfloat16
    x = np.asarray(inputs, dtype=np.float32)
    wq = np.asarray(wq, dtype=np.float32)
    wk = np.asarray(wk, dtype=np.float32)
    wv = np.asarray(wv, dtype=np.float32)
    wo = np.asarray(wo, dtype=np.float32)
    bq = np.asarray(bq, dtype=np.float32)
    bk = np.asarray(bk, dtype=np.float32)
    mask2d = np.asarray(mask, dtype=np.float32).reshape(S, S)
    causal_ref = 1.0 - np.tril(np.ones((S, S), dtype=np.float32))
    causal = bool(np.array_equal(mask2d, causal_ref))
    variant = "causal" if causal else "generic"
    if variant not in _CACHE:
        _CACHE[variant] = _build_causal() if causal else _build_generic()
    nc = _CACHE[variant]

    wdt = bf16 if causal else np.float32

    in_maps = []
    for c in range(NCORES):
        b, hg = c // 2, c % 2
        sl = slice(512 * hg, 512 * hg + 512)
        m = {
            "xt": np.ascontiguousarray(
                x[b].T.reshape(EC, 128, S).transpose(1, 0, 2)).astype(wdt),
            "wq": np.ascontiguousarray(
                wq[:, sl].reshape(EC, 128, 512).transpose(1, 0, 2)).astype(wdt),
            "wk": np.ascontiguousarray(
                wk[:, sl].reshape(EC, 128, 512).transpose(1, 0, 2)).astype(wdt),
            "wv": np.ascontiguousarray(
                wv[:, sl].reshape(EC, 128, 512).transpose(1, 0, 2)).astype(wdt),
            "wo": np.ascontiguousarray(
                wo.reshape(EC, 128, 1024).transpose(1, 0, 2)).astype(wdt),
            "bq": np.ascontiguousarray(bq[sl].reshape(4, 128).T),
            "bk": np.ascontiguousarray(bk[sl].reshape(4, 128).T),
        }
        if not causal:
            m["mkt"] = np.ascontiguousarray(
                (mask2d.T * np.float32(-2e9)).reshape(TT, 128, S).transpose(1, 0, 2))
        in_maps.append(m)

    res = run_bass_kernel_spmd(nc, in_maps, core_ids=list(range(NCORES)))
    full = np.empty((B, S, E), dtype=np.float32)
    for c in range(NCORES):
        b, hg = c // 2, c % 2
        full[b, 512 * hg:512 * hg + 512, :] = res.results[c]["out"].reshape(512, 1024)

    # biases bv/bo are zero in this problem; fold in exactly if ever nonzero.
    bv = np.asarray(bv, dtype=np.float32)
    bo = np.asarray(bo, dtype=np.float32)
    if np.any(bv != 0):
        # z_norm[b,h,s,d] += bv[64h+d]  =>  X2 += Bmat  =>  out += Bmat @ wo
        bmat = np.zeros((S, E), dtype=np.float64)
        tpr = np.arange(S)
        e = np.arange(E)
        bmat[:, :] = bv[(64 * (tpr[:, None] // 64) + e[None, :] % 64)]
        full += (bmat @ np.asarray(wo, dtype=np.float64)).astype(np.float32)[None]
    if np.any(bo != 0):
        full += bo[None, None, :]
    return full
